# revision 5
# baseline (speedup 1.0000x reference)
"""Trainium2 Bass kernel for nn_DenseAttentionOneHead.

reference:  q = X @ W^T ; pre = q @ X^T ; out = pre @ X     (X [2,4096,1024])
All linear, so out_b = (X_b W^T)(X_b^T X_b) = Q_b S_b with
  Q_b = X_b W^T,  S_b = X_b^T X_b  ([D,D], summed over rows -> AllReduce).

Sharding (8 cores): cores 0-3 handle batch 0, cores 4-7 batch 1; each core owns
1024 rows of its batch.  Per core (Plan A — fill the AllReduce window):
  S_part = Xs^T Xs  (column-half at a time, chunk-outer so PE starts early)
     -> fp16 -> AllReduce per column half (two pipelined collectives)
  Wt^T, Xs^T  (PE transposes) and Q^T = W Xs^T run while the AllReduces fly
  out = Q S   (lhsT = Q^T blocks, rhs = S), column-half at a time
"""

import numpy as np

import concourse.mybir as mybir
import concourse.tile as tile
from concourse import bacc
from concourse.bass_utils import run_bass_kernel_spmd
from concourse.masks import make_identity

F32 = mybir.dt.float32
F32R = mybir.dt.float32r
F16 = mybir.dt.float16
P = 128
D = 1024
B = 2
N = 4096
NCORES = 8
GROUP = 4          # cores per batch
ROWS = N // GROUP  # 1024 rows per core
NO = D // P        # 8
RO = ROWS // P     # 8 row-chunks per core
H = 512            # column half width
WAVE = 4           # S-phase PSUM accumulators in flight

_compiled = None


def _build():
    nc = bacc.Bacc(None, target_bir_lowering=False, debug=False, num_devices=NCORES)

    xs = nc.dram_tensor("xs", [ROWS, D], F32, kind="ExternalInput")
    w = nc.dram_tensor("w", [D, D], F32, kind="ExternalInput")
    o_out = nc.dram_tensor("o_out", [ROWS, D], F32, kind="ExternalOutput")

    s_bounce = [nc.dram_tensor(f"s_bounce{h}", [D, H], F16) for h in range(2)]
    s_red = [nc.dram_tensor(f"s_red{h}", [D, H], F16) for h in range(2)]

    with tile.TileContext(nc) as tc:
        with (
            tc.tile_pool(name="big", bufs=1) as big,
            tc.tile_pool(name="wstage", bufs=3) as wstage,
            tc.tile_pool(name="stage", bufs=4) as stage,
            tc.tile_pool(name="psum", bufs=WAVE, space="PSUM") as psum,
            tc.tile_pool(name="psum_t", bufs=2, space="PSUM") as psum_t,
        ):
            A = big.tile([P, RO, D], F32R, tag="A")        # Xs, row-chunk layout
            WT = big.tile([P, NO, D], F32R, tag="WT")      # W^T  [d, e]
            At = big.tile([P, NO, ROWS], F32R, tag="At")   # Xs^T [d, n]
            Qt = big.tile([P, NO, ROWS], F32R, tag="Qt")   # Q^T  [e, n]
            S = big.tile([P, NO, D], F32R, tag="A")        # reuses A's buffer

            ident_f = stage.tile([P, P], F32, tag="ident_f")
            make_identity(nc, ident_f)
            ident = stage.tile([P, P], F32R, tag="ident")
            nc.vector.tensor_copy(ident[:], ident_f[:])

            # Per-chunk loads so the first matmuls start early
            for ch in range(RO):
                nc.sync.dma_start(
                    A[:, ch, :], xs[ch * P : (ch + 1) * P, :].bitcast(F32R)
                )

            # ---- S_part = Xs^T Xs, one column half at a time, chunk-outer
            for h in range(2):
                for wv in range(NO // WAVE):
                    accs = [
                        psum.tile([P, H], F32, tag="acc", name=f"acc_{h}_{wv}_{wi}")
                        for wi in range(WAVE)
                    ]
                    for ch in range(RO):
                        for wi in range(WAVE):
                            et = wv * WAVE + wi
                            nc.tensor.matmul(
                                accs[wi][:],
                                A[:, ch, et * P : (et + 1) * P],
                                A[:, ch, h * H : (h + 1) * H],
                                start=(ch == 0),
                                stop=(ch == RO - 1),
                            )
                    for wi in range(WAVE):
                        et = wv * WAVE + wi
                        sh = stage.tile([P, H], F16, tag="sh")
                        nc.vector.tensor_copy(sh[:], accs[wi][:])
                        nc.sync.dma_start(
                            s_bounce[h][et * P : (et + 1) * P, :], sh[:]
                        )
                # AllReduce this column half over the 4-core group
                nc.gpsimd.collective_compute(
                    "AllReduce",
                    mybir.AluOpType.add,
                    replica_groups=[[0, 1, 2, 3], [4, 5, 6, 7]],
                    ins=[s_bounce[h][:].opt()],
                    outs=[s_red[h][:].opt()],
                )

            # ---- W^T and Xs^T transposes + Q^T = W Xs^T (fill the AR window)
            for eo in range(NO):
                wst = wstage.tile([P, D], F32R, tag="wst")
                nc.sync.dma_start(wst[:], w[eo * P : (eo + 1) * P, :].bitcast(F32R))
                for do in range(NO):
                    pt = psum_t.tile([P, P], F32R, tag="pt")
                    nc.tensor.transpose(pt[:], wst[:, do * P : (do + 1) * P], ident[:])
                    nc.scalar.copy(WT[:, do, eo * P : (eo + 1) * P], pt[:])
            for no in range(RO):
                for do in range(NO):
                    pt = psum_t.tile([P, P], F32R, tag="pt")
                    nc.tensor.transpose(pt[:], A[:, no, do * P : (do + 1) * P], ident[:])
                    nc.scalar.copy(At[:, do, no * P : (no + 1) * P], pt[:])

            # Q^T[e, n] = sum_d W[e, d] Xs[n, d] : lhsT = W^T blocks, rhs = Xs^T
            for et in range(NO):
                for h in range(2):
                    acc = psum.tile([P, H], F32, tag="acc")
                    for ch in range(NO):
                        nc.tensor.matmul(
                            acc[:],
                            WT[:, ch, et * P : (et + 1) * P],
                            At[:, ch, h * H : (h + 1) * H],
                            start=(ch == 0),
                            stop=(ch == NO - 1),
                        )
                    nc.vector.tensor_copy(Qt[:, et, h * H : (h + 1) * H], acc[:])

            # ---- per half: S back from the collective, upcast, then out columns
            for h in range(2):
                for eo in range(NO):
                    sr = stage.tile([P, H], F16, tag="sr")
                    nc.sync.dma_start(sr[:], s_red[h][eo * P : (eo + 1) * P, :])
                    nc.vector.tensor_copy(S[:, eo, h * H : (h + 1) * H], sr[:])

                # out[:, h] = Q S[:, h] : lhsT = Q^T blocks, rhs = S half
                for nt in range(RO):
                    acc = psum.tile([P, H], F32, tag="acc")
                    for ch in range(NO):
                        nc.tensor.matmul(
                            acc[:],
                            Qt[:, ch, nt * P : (nt + 1) * P],
                            S[:, ch, h * H : (h + 1) * H],
                            start=(ch == 0),
                            stop=(ch == NO - 1),
                        )
                    ot = stage.tile([P, H], F32, tag="ot")
                    nc.vector.tensor_copy(ot[:], acc[:])
                    nc.sync.dma_start(
                        o_out[nt * P : (nt + 1) * P, h * H : (h + 1) * H], ot[:]
                    )

    nc.finalize()
    return nc


def _get_compiled():
    global _compiled
    if _compiled is None:
        _compiled = _build()
    return _compiled


def kernel(hidden_states, queries, _trace=False, _trace_cores=None):
    x = np.ascontiguousarray(np.asarray(hidden_states, dtype=np.float32))
    w = np.ascontiguousarray(np.asarray(queries, dtype=np.float32))
    assert x.shape == (B, N, D) and w.shape == (D, D)

    nc = _get_compiled()
    in_maps = []
    for c in range(NCORES):
        b, r = c // GROUP, c % GROUP
        in_maps.append({"xs": x[b, r * ROWS : (r + 1) * ROWS], "w": w})

    res = run_bass_kernel_spmd(
        nc,
        in_maps,
        core_ids=list(range(NCORES)),
        trace=_trace,
        trace_cores=_trace_cores,
    )

    out = np.empty((B, N, D), dtype=np.float32)
    for c in range(NCORES):
        b, r = c // GROUP, c % GROUP
        out[b, r * ROWS : (r + 1) * ROWS] = res.results[c]["o_out"]

    if _trace:
        kernel.last_result = res
    return out


# revision 6
# speedup vs baseline: 1.1956x; 1.1956x over previous
"""Trainium2 Bass kernel for nn_DenseAttentionOneHead.

reference:  q = X @ W^T ; pre = q @ X^T ; out = pre @ X     (X [2,4096,1024])
All linear, so out_b = (X_b W^T)(X_b^T X_b) = Q_b S_b with
  Q_b = X_b W^T,  S_b = X_b^T X_b  ([D,D], summed over rows -> AllReduce).

Sharding (8 cores): cores 0-3 handle batch 0, cores 4-7 batch 1; each core owns
1024 rows of its batch.  The host passes each core its shard Xs, the transposed
shard Xs^T and W^T (host-side transposes are free), so the device does pure
matmul work:
  S_part = Xs^T Xs  (column-half at a time, chunk-outer so PE starts early)
     -> fp16 -> AllReduce per column half (two pipelined collectives)
  Q^T = W Xs^T      (lhsT = W^T blocks, rhs = Xs^T; fills the AR window)
  out = Q S         (lhsT = Q^T blocks, rhs = S), column-half at a time
"""

import numpy as np

import concourse.mybir as mybir
import concourse.tile as tile
from concourse import bacc
from concourse.bass_utils import run_bass_kernel_spmd

F32 = mybir.dt.float32
F32R = mybir.dt.float32r
F16 = mybir.dt.float16
P = 128
D = 1024
B = 2
N = 4096
NCORES = 8
GROUP = 4          # cores per batch
ROWS = N // GROUP  # 1024 rows per core
NO = D // P        # 8
RO = ROWS // P     # 8 row-chunks per core
H = 512            # column half width
WAVE = 4           # S-phase PSUM accumulators in flight

_compiled = None


def _build():
    nc = bacc.Bacc(None, target_bir_lowering=False, debug=False, num_devices=NCORES)

    xs = nc.dram_tensor("xs", [ROWS, D], F32, kind="ExternalInput")
    xst = nc.dram_tensor("xst", [D, ROWS], F32, kind="ExternalInput")
    wt = nc.dram_tensor("wt", [D, D], F32, kind="ExternalInput")
    o_out = nc.dram_tensor("o_out", [ROWS, D], F32, kind="ExternalOutput")

    s_bounce = [nc.dram_tensor(f"s_bounce{h}", [D, H], F16) for h in range(2)]
    s_red = [nc.dram_tensor(f"s_red{h}", [D, H], F16) for h in range(2)]

    with tile.TileContext(nc) as tc:
        with (
            tc.tile_pool(name="big", bufs=1) as big,
            tc.tile_pool(name="stage", bufs=4) as stage,
            tc.tile_pool(name="psum", bufs=6, space="PSUM") as psum,
        ):
            A = big.tile([P, RO, D], F32R, tag="A")        # Xs   [n, d]
            WT = big.tile([P, NO, D], F32R, tag="WT")      # W^T  [d, e]
            At = big.tile([P, NO, ROWS], F32R, tag="At")   # Xs^T [d, n]
            Qt = big.tile([P, NO, ROWS], F32R, tag="Qt")   # Q^T  [e, n]
            S = big.tile([P, NO, D], F32R, tag="A")        # reuses A's buffer

            # Per-chunk loads so the first matmuls start early
            for ch in range(RO):
                nc.sync.dma_start(
                    A[:, ch, :], xs[ch * P : (ch + 1) * P, :].bitcast(F32R)
                )
            for ch in range(NO):
                nc.sync.dma_start(
                    At[:, ch, :], xst[ch * P : (ch + 1) * P, :].bitcast(F32R)
                )
                nc.sync.dma_start(
                    WT[:, ch, :], wt[ch * P : (ch + 1) * P, :].bitcast(F32R)
                )

            # ---- S_part = Xs^T Xs, one column half at a time, chunk-outer
            for h in range(2):
                for wv in range(NO // WAVE):
                    accs = [
                        psum.tile([P, H], F32, tag="acc", name=f"acc_{h}_{wv}_{wi}")
                        for wi in range(WAVE)
                    ]
                    for ch in range(RO):
                        for wi in range(WAVE):
                            et = wv * WAVE + wi
                            nc.tensor.matmul(
                                accs[wi][:],
                                A[:, ch, et * P : (et + 1) * P],
                                A[:, ch, h * H : (h + 1) * H],
                                start=(ch == 0),
                                stop=(ch == RO - 1),
                            )
                    for wi in range(WAVE):
                        et = wv * WAVE + wi
                        sh = stage.tile([P, H], F16, tag="sh")
                        nc.vector.tensor_copy(sh[:], accs[wi][:])
                        nc.sync.dma_start(
                            s_bounce[h][et * P : (et + 1) * P, :], sh[:]
                        )
                # AllReduce this column half over the 4-core group
                nc.gpsimd.collective_compute(
                    "AllReduce",
                    mybir.AluOpType.add,
                    replica_groups=[[0, 1, 2, 3], [4, 5, 6, 7]],
                    ins=[s_bounce[h][:].opt()],
                    outs=[s_red[h][:].opt()],
                )

            # ---- Q^T[e, n] = sum_d W[e, d] Xs[n, d] (fills the AR window)
            for et in range(NO):
                for h in range(2):
                    acc = psum.tile([P, H], F32, tag="acc")
                    for ch in range(NO):
                        nc.tensor.matmul(
                            acc[:],
                            WT[:, ch, et * P : (et + 1) * P],
                            At[:, ch, h * H : (h + 1) * H],
                            start=(ch == 0),
                            stop=(ch == NO - 1),
                        )
                    nc.vector.tensor_copy(Qt[:, et, h * H : (h + 1) * H], acc[:])

            # ---- per half: S back from the collective, upcast, then out columns
            for h in range(2):
                for eo in range(NO):
                    sr = stage.tile([P, H], F16, tag="sr")
                    nc.sync.dma_start(sr[:], s_red[h][eo * P : (eo + 1) * P, :])
                    nc.vector.tensor_copy(S[:, eo, h * H : (h + 1) * H], sr[:])

                # out[:, h] = Q S[:, h] : lhsT = Q^T blocks, rhs = S half
                for nt in range(RO):
                    acc = psum.tile([P, H], F32, tag="acc")
                    for ch in range(NO):
                        nc.tensor.matmul(
                            acc[:],
                            Qt[:, ch, nt * P : (nt + 1) * P],
                            S[:, ch, h * H : (h + 1) * H],
                            start=(ch == 0),
                            stop=(ch == NO - 1),
                        )
                    ot = stage.tile([P, H], F32, tag="ot")
                    nc.vector.tensor_copy(ot[:], acc[:])
                    nc.sync.dma_start(
                        o_out[nt * P : (nt + 1) * P, h * H : (h + 1) * H], ot[:]
                    )

    nc.finalize()
    return nc


def _get_compiled():
    global _compiled
    if _compiled is None:
        _compiled = _build()
    return _compiled


def kernel(hidden_states, queries, _trace=False, _trace_cores=None):
    x = np.ascontiguousarray(np.asarray(hidden_states, dtype=np.float32))
    w = np.ascontiguousarray(np.asarray(queries, dtype=np.float32))
    assert x.shape == (B, N, D) and w.shape == (D, D)

    nc = _get_compiled()
    wt = np.ascontiguousarray(w.T)
    in_maps = []
    for c in range(NCORES):
        b, r = c // GROUP, c % GROUP
        shard = x[b, r * ROWS : (r + 1) * ROWS]
        in_maps.append(
            {"xs": shard, "xst": np.ascontiguousarray(shard.T), "wt": wt}
        )

    res = run_bass_kernel_spmd(
        nc,
        in_maps,
        core_ids=list(range(NCORES)),
        trace=_trace,
        trace_cores=_trace_cores,
    )

    out = np.empty((B, N, D), dtype=np.float32)
    for c in range(NCORES):
        b, r = c // GROUP, c % GROUP
        out[b, r * ROWS : (r + 1) * ROWS] = res.results[c]["o_out"]

    if _trace:
        kernel.last_result = res
    return out
